# revision 41
# baseline (speedup 1.0000x reference)
"""Multi-head attention (B=4, N=2048, DIM=1024, H=16, HD=64) on 8 TRN2 cores.

Sharding: tensor-parallel over heads — 2 heads per core. The reference omits
the output projection, so each core's output is a disjoint 128-column slice of
the final [B, N, 1024]; no collectives are needed.

Per-core device kernel (bf16 compute, fp32 PSUM accumulation):
  - QKV projection from a single pass over x^T: q^T,k^T produced transposed
    [outch, tokens] (weights stationary), v produced natural [tokens, outch]
    (x tiles stationary).
  - scores^T = k^T.T @ q^T per (batch, head): K=64 contraction; head A lives
    on partitions 0-63 and head B on 64-127, so the two heads' matmuls
    row-tile the PE array and run concurrently.
  - exp split across two engines: most kt-chunks on ScalarE (table exp),
    the rest on DVE via a Schraudolph-style bit-trick (i16 = a*s + b,
    bitcast to bf16), which lands within the error tolerance and removes
    the ScalarE bottleneck.
  - out^T = [1 | v]^T @ expT accumulated over k tiles; row 0 is the softmax
    denominator. Tail: DVE fast reciprocal straight from PSUM row 0, GpSimd
    partition-broadcast, DVE multiply, GpSimd per-partition bias add, DMA out.
  - Projection work is emitted as micro-pieces interleaved into the score
    stream so the PE fills the exp-paced gaps instead of idling.
"""

import numpy as np
import ml_dtypes

import concourse.bacc as bacc
import concourse.mybir as mybir
from concourse.bass_utils import run_bass_kernel_spmd
from concourse.tile import TileContext

B, N, DIM, H = 4, 2048, 1024, 16
HD = DIM // H
SCALE = 1.0 / np.sqrt(HD)
TOK = B * N               # 8192 tokens
NCORES = 8
HPC = H // NCORES         # heads per core = 2

BF16 = mybir.dt.bfloat16
F32 = mybir.dt.float32
I16 = mybir.dt.int16
AF = mybir.ActivationFunctionType
ALU = mybir.AluOpType


NT = TOK // 512           # 16 token tiles of 512 for the projection
KT = 8                    # 1024 / 128 contraction tiles
QT = N // 512             # 4 q tiles per (b, h)
KTOK = N // 128           # 16 k-token tiles per (b, h)
VROW = 2 * (HD + 1)       # 130: [1 | vA | 1 | vB] per token tile

# Schraudolph bf16 exp: i16 = trunc(A*s + B), bitcast to bf16.
# A = 128/ln2 (SCALE folded into wq on host). B tuned for truncation.
SCH_A = 184.6650309
SCH_B = 16248.5

# kt chunks computed on DVE (bit-trick); rest on ScalarE table exp.
DVE_CHUNKS = frozenset((2, 4, 7, 9, 12, 14))
# final batch has no projection filler; use a denser DVE split to shorten
# the exp chain that paces those waves
DVE_CHUNKS_LAST = frozenset((1, 3, 5, 7, 9, 11, 13))


def build_graph():
    nc = bacc.Bacc("TRN2", target_bir_lowering=False, debug=False)
    xt = nc.declare_dram_parameter("xt", [DIM, TOK], BF16, isOutput=False)
    wqk = nc.declare_dram_parameter("wqk", [DIM, 2 * HPC * HD], BF16, isOutput=False)
    wv = nc.declare_dram_parameter("wv", [DIM, HPC * HD], BF16, isOutput=False)
    bqk = nc.declare_dram_parameter("bqk", [2 * HPC * HD, 1], F32, isOutput=False)
    bvq = nc.declare_dram_parameter("bvq", [HD + 1, HPC], F32, isOutput=False)
    out = nc.declare_dram_parameter("out", [HPC, B, HD, N], F32, isOutput=True)
    NTB = N // 512            # 4 proj token-tiles per batch
    KTOK_B = N // 128         # 16 k-token tiles per batch

    with TileContext(nc) as tc:
        with (
            tc.tile_pool(name="const", bufs=1) as constp,
            tc.tile_pool(name="qk", bufs=1) as qkp,
            tc.tile_pool(name="xin", bufs=4) as xinp,
            tc.tile_pool(name="exps", bufs=32) as expp,
            tc.tile_pool(name="outs", bufs=6) as outp,
            tc.tile_pool(name="rcs", bufs=2) as rcp,
        ):
            # ---- first x tile + weights first: they gate the first matmul,
            # and the serial Sync trigger queue issues DMAs in program order
            xnt_tiles = {}

            def load(nt):
                xnt = xinp.tile([128, KT * 512], BF16, name="xnt")
                nc.sync.dma_start(
                    out=xnt.rearrange("p (kt j) -> p kt j", kt=KT),
                    in_=xt.rearrange("(kt p) tok -> p kt tok", p=128)[
                        :, :, nt * 512:(nt + 1) * 512])
                xnt_tiles[nt] = xnt

            # bqk first: the first q-copy needs it, and its trigger is slow
            # (128 tiny descriptors) — behind the x loads it gates wave 0
            bqk_s = constp.tile([128, 2], F32)
            for mt in range(2):
                nc.sync.dma_start(out=bqk_s[:, mt:mt + 1],
                                  in_=bqk[mt * 128:(mt + 1) * 128, :])
            wqk_s = constp.tile([128, KT * 256], BF16)
            nc.sync.dma_start(
                out=wqk_s.rearrange("p (kt j) -> p kt j", kt=KT),
                in_=wqk.rearrange("(kt p) j -> p kt j", p=128))
            # first x tile in two halves: the first qk group only reads
            # kt 0-3, so projection starts after half the transfer
            xnt0 = xinp.tile([128, KT * 512], BF16, name="xnt")
            for kh in range(2):
                nc.sync.dma_start(
                    out=xnt0.rearrange("p (kt j) -> p kt j", kt=KT)[
                        :, kh * 4:(kh + 1) * 4, :],
                    in_=xt.rearrange("(kt p) tok -> p kt tok", p=128)[
                        :, kh * 4:(kh + 1) * 4, 0:512])
            xnt_tiles[0] = xnt0
            wv_s = constp.tile([128, KT * 128], BF16)
            nc.sync.dma_start(
                out=wv_s.rearrange("p (kt j) -> p kt j", kt=KT),
                in_=wv.rearrange("(kt p) j -> p kt j", p=128))
            for nt0 in range(1, NTB):
                load(nt0)
            bvq_s = constp.tile([HD + 1, HPC], F32)
            nc.sync.dma_start(out=bvq_s[:, :], in_=bvq[:, :])

            # per-batch activation tensors (lets attention on batch b start
            # as soon as batch b's projection tiles land)
            q_sb = [qkp.tile([128, N], BF16, name=f"q_sb{_b}") for _b in range(B)]
            k_sb = [qkp.tile([128, N], BF16, name=f"k_sb{_b}") for _b in range(B)]
            v_sb = [qkp.tile([128, KTOK_B * VROW], BF16, name=f"v_sb{_b}") for _b in range(B)]
            # memsets on GpSimd (idle at startup) so DVE is free immediately;
            # warm tile first — it gates the PE clock warm-up
            warm = constp.tile([128, 128], BF16)
            nc.gpsimd.memset(warm[:, :], 0.25)
            for _b in range(B):
                nc.gpsimd.memset(v_sb[_b][:, :], 1.0)

            with (
                tc.tile_pool(name="qkps", bufs=1, space="PSUM") as qkps,
                tc.tile_pool(name="vps", bufs=1, space="PSUM") as vps,
                tc.tile_pool(name="sps", bufs=2, space="PSUM") as sps,
                tc.tile_pool(name="avps", bufs=1, space="PSUM") as avps,
            ):
                # PE p-state warm-up: ~6µs of junk matmuls keep the array
                # busy while the first x/w DMAs land, so real matmuls start
                # at full clock instead of the 0.65 GHz cold state
                wps = vps.tile([128, 128], F32, name="vp", tag="vp")
                NWARM = 72
                for wi in range(NWARM):
                    nc.tensor.matmul(wps[:, :], lhsT=warm[:, :], rhs=warm[:, :],
                                     start=(wi == 0), stop=(wi == NWARM - 1))

                qk_ps_live = {}

                # ---- projection micro-pieces -------------------------------
                # Each piece is a small closure; cost is approximate PE-µs.
                def fill_pieces(bb, with_loads=True):
                    pieces = []
                    for ntb in range(NTB):
                        nt = bb * NTB + ntb
                        if with_loads:
                            pieces.append((0.05, lambda nt=nt: load(nt),
                                           bb, ntb))

                        def qk_mm(nt, mt, kh):
                            xnt = xnt_tiles[nt]
                            if kh == 0:
                                ps = qkps.tile([128, 512], F32,
                                               name="ps", tag="ps")
                                qk_ps_live[(nt, mt)] = ps
                            else:
                                ps = qk_ps_live[(nt, mt)]
                            for kt in range(kh * 4, kh * 4 + 4):
                                nc.tensor.matmul(
                                    ps[:, :],
                                    lhsT=wqk_s[:, kt * 256 + mt * 128:
                                               kt * 256 + (mt + 1) * 128],
                                    rhs=xnt[:, kt * 512:(kt + 1) * 512],
                                    start=(kt == 0), stop=(kt == KT - 1))

                        def qk_copy(nt, mt):
                            bb2, ntb2 = nt // NTB, nt % NTB
                            ps = qk_ps_live.pop((nt, mt))
                            dst = q_sb[bb2] if mt == 0 else k_sb[bb2]
                            nc.vector.tensor_scalar_add(
                                dst[:, ntb2 * 512:(ntb2 + 1) * 512], ps[:, :],
                                bqk_s[:, mt:mt + 1])

                        def v_mm(nt, sub):
                            bb2, ntb2 = nt // NTB, nt % NTB
                            xnt = xnt_tiles[nt]
                            ttb = ntb2 * 4 + sub
                            vp = vps.tile([128, 128], F32, name="vp", tag="vp")
                            for kt in range(KT):
                                nc.tensor.matmul(
                                    vp[:, :],
                                    lhsT=xnt[:, kt * 512 + sub * 128:
                                             kt * 512 + (sub + 1) * 128],
                                    rhs=wv_s[:, kt * 128:(kt + 1) * 128],
                                    start=(kt == 0), stop=(kt == KT - 1))
                            # both heads in one strided copy:
                            # [128, 2, 64] -> v_sb cols [blk+1:blk+65],[blk+66:blk+130]
                            nc.vector.tensor_copy(
                                v_sb[bb2][:, ttb * VROW:(ttb + 1) * VROW]
                                .rearrange("p (h c) -> p h c", h=2)[:, :, 1:HD + 1],
                                vp.rearrange("p (h c) -> p h c", h=2))

                        def piece(cost, fn, *a):
                            return (cost, lambda fn=fn, a=a: fn(*a), bb, ntb)

                        # v pieces spaced between the two qk groups so the
                        # qkps buffer's DVE copy has drained before reuse
                        pieces += [
                            piece(0.9, qk_mm, nt, 0, 0),
                            piece(0.9, qk_mm, nt, 0, 1),
                            piece(0.0, qk_copy, nt, 0),
                            piece(0.5, v_mm, nt, 0),
                            piece(0.5, v_mm, nt, 1),
                            piece(0.9, qk_mm, nt, 1, 0),
                            piece(0.9, qk_mm, nt, 1, 1),
                            piece(0.0, qk_copy, nt, 1),
                            piece(0.5, v_mm, nt, 2),
                            piece(0.5, v_mm, nt, 3),
                        ]
                    return pieces

                def emit_tail(pb, pqt, av):
                    # batch per engine so DVE never head-of-line blocks on
                    # a gpsimd broadcast
                    rcs, bcss, ots = [], [], []
                    for h in range(2):
                        rc = rcp.tile([1, 512], F32, name=f"rc{h}", tag=f"rc{h}")
                        nc.vector.reciprocal_approx_fast(rc[0:1, :], av[h][0:1, :])
                        rcs.append(rc)
                    for h in range(2):
                        bcs = rcp.tile([65, 512], F32, name=f"bcs{h}", tag=f"bcs{h}")
                        nc.gpsimd.partition_broadcast(bcs[:, :], rcs[h][0:1, :])
                        bcss.append(bcs)
                    for h in range(2):
                        ot = outp.tile([65, 512], F32)
                        nc.vector.tensor_mul(ot[0:65, :], av[h][0:65, :],
                                             bcss[h][0:65, :])
                        ots.append(ot)
                    for h in range(2):
                        ot2 = outp.tile([65, 512], F32, name="ot2", tag="ot2")
                        nc.scalar.activation(ot2[0:65, :], ots[h][0:65, :],
                                             AF.Identity, bias=bvq_s[:, h:h + 1])
                        nc.sync.dma_start(
                            out=out[h, pb, :, pqt * 512:(pqt + 1) * 512],
                            in_=ot2[1:65, :])

                from collections import deque
                filler = deque()
                credit = [0.0]

                def pop_fill(add):
                    credit[0] += add
                    while filler and credit[0] > 0:
                        cost, fn, _, _ = filler.popleft()
                        credit[0] -= cost
                        fn()
                    if not filler:
                        credit[0] = 0.0

                def ensure_proj(bb, upto_ntb):
                    # scores of wave (bb, 0) chunk kt read k_sb[bb] columns
                    # written by block kt//4 — emission order IS the data
                    # order for the tile scheduler, so force-drain those
                    # pieces before emitting the consumer
                    while filler and any(
                            p[2] == bb and p[3] <= upto_ntb for p in filler):
                        cost, fn, _, _ = filler.popleft()
                        credit[0] -= cost
                        fn()

                # batch 0: x-loads were issued with the constants; nt0's
                # compute runs inline, nt1-3 compute becomes wave-0 filler
                b0_pieces = fill_pieces(0, with_loads=False)
                for cost, fn, _, _ in b0_pieces[:10]:    # nt0 compute
                    fn()
                filler.extend(b0_pieces[10:])

                for b in range(B):
                    for qt in range(QT):
                        if qt == 1 and b + 1 < B:
                            filler.extend(fill_pieces(b + 1))
                        qcol = qt * 512
                        dve_set = DVE_CHUNKS_LAST if b == B - 1 else DVE_CHUNKS
                        pav = [avps.tile([65, 512], F32, name=f"av{_h}",
                                         tag=f"av{_h}", bufs=1)
                               for _h in range(2)]
                        echunks = []
                        for kt in range(KTOK_B):
                            kcol = kt * 128
                            if qt == 0:
                                ensure_proj(b, kt // 4)
                            s2 = sps.tile([128, 1024], F32, name="s2", tag="s2")
                            for h in range(2):
                                nc.tensor.matmul(
                                    s2[:, h * 512:(h + 1) * 512],
                                    lhsT=k_sb[b][h * 64:(h + 1) * 64, kcol:kcol + 128],
                                    rhs=q_sb[b][h * 64:(h + 1) * 64, qcol:qcol + 512],
                                    start=True, stop=True,
                                    tile_position=(h * 64, 0))
                            e2 = expp.tile([128, 1024], BF16, name="e2", tag="e2")
                            if kt in dve_set:
                                nc.vector.tensor_scalar(
                                    out=e2[:, :].bitcast(I16), in0=s2[:, :],
                                    scalar1=SCH_A, scalar2=SCH_B,
                                    op0=ALU.mult, op1=ALU.add)
                            else:
                                nc.scalar.activation(e2[:, :], s2[:, :], AF.Exp)
                            echunks.append(e2)
                            # keep PE busy while the exp engines drain scores
                            gap_budget = 1.5 if (b, qt) == (0, 0) else 0.35
                            pop_fill(gap_budget if kt < KTOK_B - 1 else 0.0)
                        budget = 0.6 if (b, qt) != (B - 1, QT - 1) else 1e9
                        pop_fill(budget)
                        # kt-major so AV consumption tracks exp production
                        for kt in range(KTOK_B):
                            for h in range(2):
                                nc.tensor.matmul(
                                    pav[h][:, :],
                                    lhsT=v_sb[b][:, kt * VROW + h * (HD + 1):
                                                 kt * VROW + (h + 1) * (HD + 1)],
                                    rhs=echunks[kt][:, h * 512:(h + 1) * 512],
                                    start=(kt == 0), stop=(kt == KTOK_B - 1),
                                    skip_group_check=True)
                        emit_tail(b, qt, pav)
    nc.compile()
    return nc


_GRAPH = None


def _get_graph():
    global _GRAPH
    if _GRAPH is None:
        _GRAPH = build_graph()
    return _GRAPH


def _make_in_maps(x, w_qkv, b_qkv):
    bf = ml_dtypes.bfloat16
    xt = np.ascontiguousarray(x.reshape(TOK, DIM).T).astype(bf)
    in_maps = []
    for c in range(NCORES):
        hA, hB = HPC * c, HPC * c + 1
        rq = [w_qkv[h * HD:(h + 1) * HD] * SCALE for h in (hA, hB)]
        rk = [w_qkv[DIM + h * HD: DIM + (h + 1) * HD] for h in (hA, hB)]
        rv = [w_qkv[2 * DIM + h * HD: 2 * DIM + (h + 1) * HD] for h in (hA, hB)]
        wqk_c = np.ascontiguousarray(np.concatenate(rq + rk, axis=0).T).astype(bf)
        wv_c = np.ascontiguousarray(np.concatenate(rv, axis=0).T).astype(bf)
        bq = [b_qkv[h * HD:(h + 1) * HD] * SCALE for h in (hA, hB)]
        bk = [b_qkv[DIM + h * HD: DIM + (h + 1) * HD] for h in (hA, hB)]
        bvc = [b_qkv[2 * DIM + h * HD: 2 * DIM + (h + 1) * HD] for h in (hA, hB)]
        bqk_c = np.concatenate(bq + bk).astype(np.float32).reshape(-1, 1)
        bvq_c = np.zeros((HD + 1, HPC), dtype=np.float32)
        for hh in range(HPC):
            bvq_c[1:HD + 1, hh] = bvc[hh]
        in_maps.append({"xt": xt, "wqk": wqk_c, "wv": wv_c,
                        "bqk": np.ascontiguousarray(bqk_c),
                        "bvq": bvq_c})
    return in_maps


def _run(x, w_qkv, b_qkv, trace=False, tmpdir=None):
    nc = _get_graph()
    in_maps = _make_in_maps(np.asarray(x, dtype=np.float32),
                            np.asarray(w_qkv, dtype=np.float32),
                            np.asarray(b_qkv, dtype=np.float32))
    res = run_bass_kernel_spmd(nc, in_maps, core_ids=list(range(NCORES)),
                               trace=trace, tmpdir=tmpdir)
    full = np.empty((B, N, DIM), dtype=np.float32)
    for c in range(NCORES):
        oc = res.results[c]["out"]          # [HPC, B, HD, N]
        # out[b, q, (HPC*c+hh)*HD + d] = oc[hh, b, d, q]
        full[:, :, c * HPC * HD:(c + 1) * HPC * HD] = \
            oc.transpose(1, 3, 0, 2).reshape(B, N, HPC * HD)
    return full, res


def kernel(x, w_qkv, b_qkv):
    full, _ = _run(x, w_qkv, b_qkv, trace=False)
    return full


# revision 43
# speedup vs baseline: 1.0081x; 1.0081x over previous
"""Multi-head attention (B=4, N=2048, DIM=1024, H=16, HD=64) on 8 TRN2 cores.

Sharding: tensor-parallel over heads — 2 heads per core. The reference omits
the output projection, so each core's output is a disjoint 128-column slice of
the final [B, N, 1024]; no collectives are needed.

Per-core device kernel (bf16 compute, fp32 PSUM accumulation):
  - QKV projection from a single pass over x^T: q^T,k^T produced transposed
    [outch, tokens] (weights stationary), v produced natural [tokens, outch]
    (x tiles stationary).
  - scores^T = k^T.T @ q^T per (batch, head): K=64 contraction; head A lives
    on partitions 0-63 and head B on 64-127, so the two heads' matmuls
    row-tile the PE array and run concurrently.
  - exp split across two engines: most kt-chunks on ScalarE (table exp),
    the rest on DVE via a Schraudolph-style bit-trick (i16 = a*s + b,
    bitcast to bf16), which lands within the error tolerance and removes
    the ScalarE bottleneck.
  - out^T = [1 | v]^T @ expT accumulated over k tiles; row 0 is the softmax
    denominator. Tail: DVE fast reciprocal straight from PSUM row 0, GpSimd
    partition-broadcast, DVE multiply, GpSimd per-partition bias add, DMA out.
  - Projection work is emitted as micro-pieces interleaved into the score
    stream so the PE fills the exp-paced gaps instead of idling.
"""

import numpy as np
import ml_dtypes

import concourse.bacc as bacc
import concourse.mybir as mybir
from concourse.bass_utils import run_bass_kernel_spmd
from concourse.tile import TileContext

B, N, DIM, H = 4, 2048, 1024, 16
HD = DIM // H
SCALE = 1.0 / np.sqrt(HD)
TOK = B * N               # 8192 tokens
NCORES = 8
HPC = H // NCORES         # heads per core = 2

BF16 = mybir.dt.bfloat16
F32 = mybir.dt.float32
I16 = mybir.dt.int16
AF = mybir.ActivationFunctionType
ALU = mybir.AluOpType


NT = TOK // 512           # 16 token tiles of 512 for the projection
KT = 8                    # 1024 / 128 contraction tiles
QT = N // 512             # 4 q tiles per (b, h)
KTOK = N // 128           # 16 k-token tiles per (b, h)
VROW = 2 * (HD + 1)       # 130: [1 | vA | 1 | vB] per token tile

# Schraudolph bf16 exp: i16 = trunc(A*s + B), bitcast to bf16.
# A = 128/ln2 (SCALE folded into wq on host). B tuned for truncation.
SCH_A = 184.6650309
SCH_B = 16248.5

# kt chunks computed on DVE (bit-trick); rest on ScalarE table exp.
DVE_CHUNKS = frozenset((2, 4, 7, 9, 12, 14))
# final batch has no projection filler; use a denser DVE split to shorten
# the exp chain that paces those waves
DVE_CHUNKS_LAST = frozenset((1, 3, 5, 7, 9, 11, 13, 15))


def build_graph():
    nc = bacc.Bacc("TRN2", target_bir_lowering=False, debug=False)
    xt = nc.declare_dram_parameter("xt", [DIM, TOK], BF16, isOutput=False)
    wqk = nc.declare_dram_parameter("wqk", [DIM, 2 * HPC * HD], BF16, isOutput=False)
    wv = nc.declare_dram_parameter("wv", [DIM, HPC * HD], BF16, isOutput=False)
    bqk = nc.declare_dram_parameter("bqk", [2 * HPC * HD, 1], F32, isOutput=False)
    bvq = nc.declare_dram_parameter("bvq", [HD + 1, HPC], F32, isOutput=False)
    out = nc.declare_dram_parameter("out", [HPC, B, HD, N], F32, isOutput=True)
    NTB = N // 512            # 4 proj token-tiles per batch
    KTOK_B = N // 128         # 16 k-token tiles per batch

    with TileContext(nc) as tc:
        with (
            tc.tile_pool(name="const", bufs=1) as constp,
            tc.tile_pool(name="qk", bufs=1) as qkp,
            tc.tile_pool(name="xin", bufs=4) as xinp,
            tc.tile_pool(name="exps", bufs=32) as expp,
            tc.tile_pool(name="outs", bufs=6) as outp,
            tc.tile_pool(name="rcs", bufs=2) as rcp,
        ):
            # ---- first x tile + weights first: they gate the first matmul,
            # and the serial Sync trigger queue issues DMAs in program order
            xnt_tiles = {}

            def load(nt):
                xnt = xinp.tile([128, KT * 512], BF16, name="xnt")
                nc.sync.dma_start(
                    out=xnt.rearrange("p (kt j) -> p kt j", kt=KT),
                    in_=xt.rearrange("(kt p) tok -> p kt tok", p=128)[
                        :, :, nt * 512:(nt + 1) * 512])
                xnt_tiles[nt] = xnt

            # bqk first: the first q-copy needs it, and its trigger is slow
            # (128 tiny descriptors) — behind the x loads it gates wave 0
            bqk_s = constp.tile([128, 2], F32)
            for mt in range(2):
                nc.sync.dma_start(out=bqk_s[:, mt:mt + 1],
                                  in_=bqk[mt * 128:(mt + 1) * 128, :])
            wqk_s = constp.tile([128, KT * 256], BF16)
            nc.sync.dma_start(
                out=wqk_s.rearrange("p (kt j) -> p kt j", kt=KT),
                in_=wqk.rearrange("(kt p) j -> p kt j", p=128))
            # first x tile in two halves: the first qk group only reads
            # kt 0-3, so projection starts after half the transfer
            xnt0 = xinp.tile([128, KT * 512], BF16, name="xnt")
            for kh in range(2):
                nc.sync.dma_start(
                    out=xnt0.rearrange("p (kt j) -> p kt j", kt=KT)[
                        :, kh * 4:(kh + 1) * 4, :],
                    in_=xt.rearrange("(kt p) tok -> p kt tok", p=128)[
                        :, kh * 4:(kh + 1) * 4, 0:512])
            xnt_tiles[0] = xnt0
            wv_s = constp.tile([128, KT * 128], BF16)
            nc.sync.dma_start(
                out=wv_s.rearrange("p (kt j) -> p kt j", kt=KT),
                in_=wv.rearrange("(kt p) j -> p kt j", p=128))
            for nt0 in range(1, NTB):
                load(nt0)
            bvq_s = constp.tile([HD + 1, HPC], F32)
            nc.sync.dma_start(out=bvq_s[:, :], in_=bvq[:, :])

            # per-batch activation tensors (lets attention on batch b start
            # as soon as batch b's projection tiles land)
            q_sb = [qkp.tile([128, N], BF16, name=f"q_sb{_b}") for _b in range(B)]
            k_sb = [qkp.tile([128, N], BF16, name=f"k_sb{_b}") for _b in range(B)]
            v_sb = [qkp.tile([128, KTOK_B * VROW], BF16, name=f"v_sb{_b}") for _b in range(B)]
            # memsets on GpSimd (idle at startup) so DVE is free immediately;
            # warm tile first — it gates the PE clock warm-up
            warm = constp.tile([128, 128], BF16)
            nc.gpsimd.memset(warm[:, :], 0.25)
            for _b in range(B):
                nc.gpsimd.memset(v_sb[_b][:, :], 1.0)

            with (
                tc.tile_pool(name="qkps", bufs=1, space="PSUM") as qkps,
                tc.tile_pool(name="vps", bufs=1, space="PSUM") as vps,
                tc.tile_pool(name="sps", bufs=2, space="PSUM") as sps,
                tc.tile_pool(name="avps", bufs=1, space="PSUM") as avps,
            ):
                # PE p-state warm-up: ~6µs of junk matmuls keep the array
                # busy while the first x/w DMAs land, so real matmuls start
                # at full clock instead of the 0.65 GHz cold state
                wps = vps.tile([128, 128], F32, name="vp", tag="vp")
                NWARM = 72
                for wi in range(NWARM):
                    nc.tensor.matmul(wps[:, :], lhsT=warm[:, :], rhs=warm[:, :],
                                     start=(wi == 0), stop=(wi == NWARM - 1))

                qk_ps_live = {}

                # ---- projection micro-pieces -------------------------------
                # Each piece is a small closure; cost is approximate PE-µs.
                def fill_pieces(bb, with_loads=True):
                    pieces = []
                    for ntb in range(NTB):
                        nt = bb * NTB + ntb
                        if with_loads:
                            pieces.append((0.05, lambda nt=nt: load(nt),
                                           bb, ntb))

                        def qk_mm(nt, mt, kh):
                            xnt = xnt_tiles[nt]
                            if kh == 0:
                                ps = qkps.tile([128, 512], F32,
                                               name="ps", tag="ps")
                                qk_ps_live[(nt, mt)] = ps
                            else:
                                ps = qk_ps_live[(nt, mt)]
                            for kt in range(kh * 4, kh * 4 + 4):
                                nc.tensor.matmul(
                                    ps[:, :],
                                    lhsT=wqk_s[:, kt * 256 + mt * 128:
                                               kt * 256 + (mt + 1) * 128],
                                    rhs=xnt[:, kt * 512:(kt + 1) * 512],
                                    start=(kt == 0), stop=(kt == KT - 1))

                        def qk_copy(nt, mt):
                            bb2, ntb2 = nt // NTB, nt % NTB
                            ps = qk_ps_live.pop((nt, mt))
                            dst = q_sb[bb2] if mt == 0 else k_sb[bb2]
                            nc.vector.tensor_scalar_add(
                                dst[:, ntb2 * 512:(ntb2 + 1) * 512], ps[:, :],
                                bqk_s[:, mt:mt + 1])

                        def v_mm(nt, sub):
                            bb2, ntb2 = nt // NTB, nt % NTB
                            xnt = xnt_tiles[nt]
                            ttb = ntb2 * 4 + sub
                            vp = vps.tile([128, 128], F32, name="vp", tag="vp")
                            for kt in range(KT):
                                nc.tensor.matmul(
                                    vp[:, :],
                                    lhsT=xnt[:, kt * 512 + sub * 128:
                                             kt * 512 + (sub + 1) * 128],
                                    rhs=wv_s[:, kt * 128:(kt + 1) * 128],
                                    start=(kt == 0), stop=(kt == KT - 1))
                            # both heads in one strided copy:
                            # [128, 2, 64] -> v_sb cols [blk+1:blk+65],[blk+66:blk+130]
                            nc.vector.tensor_copy(
                                v_sb[bb2][:, ttb * VROW:(ttb + 1) * VROW]
                                .rearrange("p (h c) -> p h c", h=2)[:, :, 1:HD + 1],
                                vp.rearrange("p (h c) -> p h c", h=2))

                        def piece(cost, fn, *a):
                            return (cost, lambda fn=fn, a=a: fn(*a), bb, ntb)

                        # v pieces spaced between the two qk groups so the
                        # qkps buffer's DVE copy has drained before reuse
                        pieces += [
                            piece(0.9, qk_mm, nt, 0, 0),
                            piece(0.9, qk_mm, nt, 0, 1),
                            piece(0.0, qk_copy, nt, 0),
                            piece(0.5, v_mm, nt, 0),
                            piece(0.5, v_mm, nt, 1),
                            piece(0.9, qk_mm, nt, 1, 0),
                            piece(0.9, qk_mm, nt, 1, 1),
                            piece(0.0, qk_copy, nt, 1),
                            piece(0.5, v_mm, nt, 2),
                            piece(0.5, v_mm, nt, 3),
                        ]
                    return pieces

                def emit_tail(pb, pqt, av):
                    # batch per engine so DVE never head-of-line blocks on
                    # a gpsimd broadcast
                    rcs, bcss, ots = [], [], []
                    for h in range(2):
                        rc = rcp.tile([1, 512], F32, name=f"rc{h}", tag=f"rc{h}")
                        nc.vector.reciprocal_approx_fast(rc[0:1, :], av[h][0:1, :])
                        rcs.append(rc)
                    for h in range(2):
                        bcs = rcp.tile([65, 512], F32, name=f"bcs{h}", tag=f"bcs{h}")
                        nc.gpsimd.partition_broadcast(bcs[:, :], rcs[h][0:1, :])
                        bcss.append(bcs)
                    for h in range(2):
                        ot = outp.tile([65, 512], F32)
                        nc.vector.tensor_mul(ot[0:65, :], av[h][0:65, :],
                                             bcss[h][0:65, :])
                        ots.append(ot)
                    for h in range(2):
                        ot2 = outp.tile([65, 512], F32, name="ot2", tag="ot2")
                        nc.scalar.activation(ot2[0:65, :], ots[h][0:65, :],
                                             AF.Identity, bias=bvq_s[:, h:h + 1])
                        nc.sync.dma_start(
                            out=out[h, pb, :, pqt * 512:(pqt + 1) * 512],
                            in_=ot2[1:65, :])

                from collections import deque
                filler = deque()
                credit = [0.0]

                def pop_fill(add):
                    credit[0] += add
                    while filler and credit[0] > 0:
                        cost, fn, _, _ = filler.popleft()
                        credit[0] -= cost
                        fn()
                    if not filler:
                        credit[0] = 0.0

                def ensure_proj(bb, upto_ntb):
                    # scores of wave (bb, 0) chunk kt read k_sb[bb] columns
                    # written by block kt//4 — emission order IS the data
                    # order for the tile scheduler, so force-drain those
                    # pieces before emitting the consumer
                    while filler and any(
                            p[2] == bb and p[3] <= upto_ntb for p in filler):
                        cost, fn, _, _ = filler.popleft()
                        credit[0] -= cost
                        fn()

                # batch 0: x-loads were issued with the constants. nt0's q/k
                # groups are hand-scheduled for the startup critical path:
                # k accumulates in the vps bank so it needs no wait on the
                # q-copy, and each half-group starts as its x half lands.
                qps0 = qkps.tile([128, 512], F32, name="ps", tag="ps")
                kps0 = vps.tile([128, 512], F32, name="vp", tag="vp")
                xnt00 = xnt_tiles[0]
                for kh in range(2):
                    for mt in range(2):
                        ps0 = qps0 if mt == 0 else kps0
                        for kt in range(kh * 4, kh * 4 + 4):
                            nc.tensor.matmul(
                                ps0[:, :],
                                lhsT=wqk_s[:, kt * 256 + mt * 128:
                                           kt * 256 + (mt + 1) * 128],
                                rhs=xnt00[:, kt * 512:(kt + 1) * 512],
                                start=(kt == 0), stop=(kt == KT - 1))
                for mt in range(2):
                    nc.vector.tensor_scalar_add(
                        (q_sb[0] if mt == 0 else k_sb[0])[:, 0:512],
                        (qps0 if mt == 0 else kps0)[:, :],
                        bqk_s[:, mt:mt + 1])
                b0_pieces = fill_pieces(0, with_loads=False)
                for cost, fn, _, _ in b0_pieces[3:5] + b0_pieces[8:10]:
                    fn()                                 # nt0 v pieces
                filler.extend(b0_pieces[10:])

                for b in range(B):
                    for qt in range(QT):
                        if qt == 1 and b + 1 < B:
                            filler.extend(fill_pieces(b + 1))
                        qcol = qt * 512
                        dve_set = DVE_CHUNKS_LAST if b == B - 1 else DVE_CHUNKS
                        pav = [avps.tile([65, 512], F32, name=f"av{_h}",
                                         tag=f"av{_h}", bufs=1)
                               for _h in range(2)]
                        echunks = []
                        for kt in range(KTOK_B):
                            kcol = kt * 128
                            if qt == 0:
                                ensure_proj(b, kt // 4)
                            s2 = sps.tile([128, 1024], F32, name="s2", tag="s2")
                            for h in range(2):
                                nc.tensor.matmul(
                                    s2[:, h * 512:(h + 1) * 512],
                                    lhsT=k_sb[b][h * 64:(h + 1) * 64, kcol:kcol + 128],
                                    rhs=q_sb[b][h * 64:(h + 1) * 64, qcol:qcol + 512],
                                    start=True, stop=True,
                                    tile_position=(h * 64, 0))
                            e2 = expp.tile([128, 1024], BF16, name="e2", tag="e2")
                            if kt in dve_set:
                                nc.vector.tensor_scalar(
                                    out=e2[:, :].bitcast(I16), in0=s2[:, :],
                                    scalar1=SCH_A, scalar2=SCH_B,
                                    op0=ALU.mult, op1=ALU.add)
                            else:
                                nc.scalar.activation(e2[:, :], s2[:, :], AF.Exp)
                            echunks.append(e2)
                            # keep PE busy while the exp engines drain scores
                            gap_budget = 1.5 if (b, qt) == (0, 0) else 0.35
                            pop_fill(gap_budget if kt < KTOK_B - 1 else 0.0)
                        budget = 0.6 if (b, qt) != (B - 1, QT - 1) else 1e9
                        pop_fill(budget)
                        # kt-major so AV consumption tracks exp production
                        for kt in range(KTOK_B):
                            for h in range(2):
                                nc.tensor.matmul(
                                    pav[h][:, :],
                                    lhsT=v_sb[b][:, kt * VROW + h * (HD + 1):
                                                 kt * VROW + (h + 1) * (HD + 1)],
                                    rhs=echunks[kt][:, h * 512:(h + 1) * 512],
                                    start=(kt == 0), stop=(kt == KTOK_B - 1),
                                    skip_group_check=True)
                        emit_tail(b, qt, pav)
    nc.compile()
    return nc


_GRAPH = None


def _get_graph():
    global _GRAPH
    if _GRAPH is None:
        _GRAPH = build_graph()
    return _GRAPH


def _make_in_maps(x, w_qkv, b_qkv):
    bf = ml_dtypes.bfloat16
    xt = np.ascontiguousarray(x.reshape(TOK, DIM).T).astype(bf)
    in_maps = []
    for c in range(NCORES):
        hA, hB = HPC * c, HPC * c + 1
        rq = [w_qkv[h * HD:(h + 1) * HD] * SCALE for h in (hA, hB)]
        rk = [w_qkv[DIM + h * HD: DIM + (h + 1) * HD] for h in (hA, hB)]
        rv = [w_qkv[2 * DIM + h * HD: 2 * DIM + (h + 1) * HD] for h in (hA, hB)]
        wqk_c = np.ascontiguousarray(np.concatenate(rq + rk, axis=0).T).astype(bf)
        wv_c = np.ascontiguousarray(np.concatenate(rv, axis=0).T).astype(bf)
        bq = [b_qkv[h * HD:(h + 1) * HD] * SCALE for h in (hA, hB)]
        bk = [b_qkv[DIM + h * HD: DIM + (h + 1) * HD] for h in (hA, hB)]
        bvc = [b_qkv[2 * DIM + h * HD: 2 * DIM + (h + 1) * HD] for h in (hA, hB)]
        bqk_c = np.concatenate(bq + bk).astype(np.float32).reshape(-1, 1)
        bvq_c = np.zeros((HD + 1, HPC), dtype=np.float32)
        for hh in range(HPC):
            bvq_c[1:HD + 1, hh] = bvc[hh]
        in_maps.append({"xt": xt, "wqk": wqk_c, "wv": wv_c,
                        "bqk": np.ascontiguousarray(bqk_c),
                        "bvq": bvq_c})
    return in_maps


def _run(x, w_qkv, b_qkv, trace=False, tmpdir=None):
    nc = _get_graph()
    in_maps = _make_in_maps(np.asarray(x, dtype=np.float32),
                            np.asarray(w_qkv, dtype=np.float32),
                            np.asarray(b_qkv, dtype=np.float32))
    res = run_bass_kernel_spmd(nc, in_maps, core_ids=list(range(NCORES)),
                               trace=trace, tmpdir=tmpdir)
    full = np.empty((B, N, DIM), dtype=np.float32)
    for c in range(NCORES):
        oc = res.results[c]["out"]          # [HPC, B, HD, N]
        # out[b, q, (HPC*c+hh)*HD + d] = oc[hh, b, d, q]
        full[:, :, c * HPC * HD:(c + 1) * HPC * HD] = \
            oc.transpose(1, 3, 0, 2).reshape(B, N, HPC * HD)
    return full, res


def kernel(x, w_qkv, b_qkv):
    full, _ = _run(x, w_qkv, b_qkv, trace=False)
    return full
